# revision 5
# baseline (speedup 1.0000x reference)
"""Competitive binding layer (fixed-point solver) on 8 TRN2 NeuronCores.

Math (reference, 64 fixed-point iterations == converged fixed point):
    K = k*k [nA,nB]; BT = bt*bt [nB]
    repeat: BF = BT/(1 + K^T @ AF); AF = AT/(1 + K @ BF)
    C = AF[:,None] * K * BF[None,:]

Strategy:
  - The 64 reference iterations fully converge; we solve for the same fixed
    point with Anderson(1)-accelerated iteration in ~8 steps.
  - K row-sharded over 8 cores (512 rows each). Each core keeps two SBUF
    layouts of its shard (f32r, full-rate PE streaming):
      krows [ip, (b, j)]   row l = ip*4+b      -> u_partial = K_loc^T @ AF_loc
      kcolsT [jp, (c, l)]  col j = jp*32+c     -> v_loc = K_loc @ BF
  - Per step: one 16KB AllReduce of the u partial sums (the only collective).
  - Anderson extrapolation runs redundantly on every core on the replicated
    u vector [128,32]; dot products complete via gpsimd partition_all_reduce.
  - Final C streamed from an exact fp32 copy of k (f32r storage is rounded).
"""
import numpy as np

N_CORES = 8
NA = 4096
NB = 4096
L = NA // N_CORES          # 512 local rows
N_LOOPS = 5                # Anderson loop count; ARs = N_LOOPS + 1

_CACHE = {}
LAST_RESULT = None


def _build():
    import concourse.bacc as bacc
    import concourse.tile as tile
    import concourse.mybir as mybir
    import concourse.bass_isa as bass_isa

    dt = mybir.dt
    nc = bacc.Bacc("TRN2", target_bir_lowering=False, debug=False,
                   num_devices=N_CORES)

    krows_d = nc.dram_tensor("krows", [128, 4 * NB], dt.float32r, kind="ExternalInput")
    kcolsT_d = nc.dram_tensor("kcolsT", [128, 32 * L], dt.float32r, kind="ExternalInput")
    kf32_d = nc.dram_tensor("kf32", [128, 4 * NB], dt.float32, kind="ExternalInput")
    at_d = nc.dram_tensor("at_sb", [128, 4], dt.float32, kind="ExternalInput")
    bt2_d = nc.dram_tensor("bt2_sb", [128, 32], dt.float32, kind="ExternalInput")
    out_d = nc.dram_tensor("cout", [128, 4 * NB], dt.float32, kind="ExternalOutput")

    with tile.TileContext(nc) as tc:
        with (
            tc.tile_pool(name="kpool", bufs=1) as kpool,
            tc.tile_pool(name="small", bufs=1) as small,
            tc.tile_pool(name="state", bufs=2) as state,
            tc.tile_pool(name="rows", bufs=3) as rows,
            tc.tile_pool(name="pu", bufs=3, space="PSUM") as pup,
            tc.tile_pool(name="pv", bufs=2, space="PSUM") as pvp,
            tc.tile_pool(name="dram", bufs=2, space="DRAM") as dram,
            tc.tile_pool(name="cph", bufs=3) as cph,
        ):
            # ---- load K shards into SBUF (chunked for DMA parallelism) ----
            krows = kpool.tile([128, 4 * NB], dt.float32r, tag="krows")
            kcolsT = kpool.tile([128, 32 * L], dt.float32r, tag="kcolsT")
            for i in range(8):
                w = 4 * NB // 8
                nc.sync.dma_start(krows[:, i * w:(i + 1) * w],
                                  krows_d[:, i * w:(i + 1) * w])
            for i in range(8):
                w = 32 * L // 8
                nc.sync.dma_start(kcolsT[:, i * w:(i + 1) * w],
                                  kcolsT_d[:, i * w:(i + 1) * w])

            at_sb = small.tile([128, 4], dt.float32, tag="at")
            bt2_sb = small.tile([128, 32], dt.float32, tag="bt2")
            nc.sync.dma_start(at_sb[:], at_d[:, :])
            nc.sync.dma_start(bt2_sb[:], bt2_d[:, :])

            ar_groups = [list(range(N_CORES))]

            def matvec1_allreduce(af_r, t):
                """u_red(dram [1,NB]) = AllReduce(krows^T @ af_r)."""
                u_part = dram.tile([1, NB], dt.float32, tag="u_part")
                u_red = dram.tile([1, NB], dt.float32, tag="u_red")
                for c8 in range(8):
                    pu = pup.tile([1, 512], dt.float32, tag="pu")
                    for b in range(4):
                        nc.tensor.matmul(
                            pu[:], af_r[:, b:b + 1],
                            krows[:, b * NB + c8 * 512: b * NB + (c8 + 1) * 512],
                            start=(b == 0), stop=(b == 3),
                        )
                    rowt = rows.tile([1, 512], dt.float32, tag="urow")
                    nc.vector.tensor_copy(rowt[:], pu[:])
                    nc.sync.dma_start(u_part[:, c8 * 512:(c8 + 1) * 512], rowt[:])
                nc.gpsimd.collective_compute(
                    "AllReduce", mybir.AluOpType.add, replica_groups=ar_groups,
                    ins=[u_part.opt()], outs=[u_red.opt()],
                )
                usb = state.tile([128, 32], dt.float32, tag=f"G{t % 3}")
                nc.sync.dma_start(
                    usb[:], u_red[:].rearrange("one (p c) -> (one p) c", p=128))
                return usb, u_red

            def bf_from_u(usb):
                """BF = BT2/(1+u): returns (f32 tile, f32r tile)."""
                bf = state.tile([128, 32], dt.float32, tag="bf")
                nc.vector.tensor_scalar_add(bf[:], usb[:], 1.0)
                nc.vector.reciprocal(bf[:], bf[:])
                nc.vector.tensor_mul(bf[:], bf[:], bt2_sb[:])
                bf_r = state.tile([128, 32], dt.float32r, tag="bfr")
                nc.vector.tensor_copy(bf_r[:], bf[:])
                return bf, bf_r

            def matvec2_af(bf_r):
                """AF = AT/(1 + kcolsT^T-contract @ bf): returns (f32, f32r)."""
                pv = pvp.tile([1, 512], dt.float32, tag="pv")
                for c in range(32):
                    nc.tensor.matmul(
                        pv[:], bf_r[:, c:c + 1],
                        kcolsT[:, c * L:(c + 1) * L],
                        start=(c == 0), stop=(c == 31),
                    )
                vrow = rows.tile([1, 512], dt.float32, tag="vrow")
                nc.vector.tensor_copy(vrow[:], pv[:])
                v_dram = dram.tile([1, 512], dt.float32, tag="vdram")
                nc.sync.dma_start(v_dram[:], vrow[:])
                vsb = state.tile([128, 4], dt.float32, tag="vsb")
                nc.sync.dma_start(
                    vsb[:], v_dram[:].rearrange("one (p c) -> (one p) c", p=128))
                af = state.tile([128, 4], dt.float32, tag="af")
                nc.vector.tensor_scalar_add(af[:], vsb[:], 1.0)
                nc.vector.reciprocal(af[:], af[:])
                nc.vector.tensor_mul(af[:], af[:], at_sb[:])
                af_r = state.tile([128, 4], dt.float32r, tag="afr")
                nc.vector.tensor_copy(af_r[:], af[:])
                return af, af_r

            # ---- initial: u_1 = AR(K^T @ AT) ----
            at_r = small.tile([128, 4], dt.float32r, tag="atr")
            nc.vector.tensor_copy(at_r[:], at_sb[:])
            u_cur, _ = matvec1_allreduce(at_r, 0)

            G_prev = None
            g_prev = None
            for t in range(1, N_LOOPS + 1):
                bf, bf_r = bf_from_u(u_cur)
                af, af_r = matvec2_af(bf_r)
                G, _ = matvec1_allreduce(af_r, t)

                g = state.tile([128, 32], dt.float32, tag=f"g{t % 3}")
                nc.vector.tensor_sub(g[:], G[:], u_cur[:])
                if t == 1:
                    u_next = G
                else:
                    dg = state.tile([128, 32], dt.float32, tag="dg")
                    nc.vector.tensor_sub(dg[:], g[:], g_prev[:])
                    t1 = state.tile([128, 32], dt.float32, tag="t1")
                    nc.vector.tensor_mul(t1[:], dg[:], dg[:])
                    t2 = state.tile([128, 32], dt.float32, tag="t2")
                    nc.vector.tensor_mul(t2[:], dg[:], g[:])
                    r1 = state.tile([128, 1], dt.float32, tag="r1")
                    r2 = state.tile([128, 1], dt.float32, tag="r2")
                    nc.vector.reduce_sum(r1[:], t1[:], axis=mybir.AxisListType.X)
                    nc.vector.reduce_sum(r2[:], t2[:], axis=mybir.AxisListType.X)
                    d1 = state.tile([128, 1], dt.float32, tag="d1")
                    d2 = state.tile([128, 1], dt.float32, tag="d2")
                    nc.gpsimd.partition_all_reduce(
                        d1[:], r1[:], channels=128, reduce_op=bass_isa.ReduceOp.add)
                    nc.gpsimd.partition_all_reduce(
                        d2[:], r2[:], channels=128, reduce_op=bass_isa.ReduceOp.add)
                    # theta = clamp(d2 / (d1 + eps), [-2, 2])  [128,1]
                    th = state.tile([128, 1], dt.float32, tag="th")
                    nc.vector.tensor_scalar_add(th[:], d1[:], 1e-30)
                    nc.vector.reciprocal(th[:], th[:])
                    nc.vector.tensor_mul(th[:], th[:], d2[:])
                    nc.vector.tensor_scalar_min(th[:], th[:], 2.0)
                    nc.vector.tensor_scalar_max(th[:], th[:], -2.0)
                    # u_next = G - th*(G - G_prev)
                    d = state.tile([128, 32], dt.float32, tag="d")
                    nc.vector.tensor_sub(d[:], G[:], G_prev[:])
                    nc.vector.tensor_scalar_mul(d[:], d[:], th[:, 0:1])
                    u_next = state.tile([128, 32], dt.float32, tag=f"un{t % 3}")
                    nc.vector.tensor_sub(u_next[:], G[:], d[:])
                G_prev = G
                g_prev = g
                u_cur = u_next

            # ---- final: BF* = BT2/(1+u*), AF* = AT/(1+K BF*), C out ----
            bf_f, bf_r = bf_from_u(u_cur)
            af_f, _ = matvec2_af(bf_r)

            # BF_rep [128, NB] fp32: bf_f -> dram (natural j) -> row -> bcast
            bf_dram = dram.tile([1, NB], dt.float32, tag="bfd")
            nc.sync.dma_start(
                bf_dram[:].rearrange("one (p c) -> (one p) c", p=128), bf_f[:])
            bf_rep = small.tile([128, NB], dt.float32, tag="bfrep")
            for q in range(4):
                bf_row = rows.tile([1, NB // 4], dt.float32, tag="bfrow")
                nc.sync.dma_start(bf_row[:], bf_dram[:, q * (NB // 4):(q + 1) * (NB // 4)])
                nc.gpsimd.partition_broadcast(
                    bf_rep[:, q * (NB // 4):(q + 1) * (NB // 4)], bf_row[:])

            HW = 1024  # C-phase streaming width
            for b in range(4):
                for h in range(4):
                    sl = slice(b * NB + h * HW, b * NB + (h + 1) * HW)
                    jl = slice(h * HW, (h + 1) * HW)
                    kf = cph.tile([128, HW], dt.float32, tag="kf")
                    nc.sync.dma_start(kf[:], kf32_d[:, sl])
                    w = cph.tile([128, HW], dt.float32, tag="w")
                    # kf32 holds K = k*k already; just scale by AF and BF
                    nc.vector.tensor_scalar_mul(w[:], kf[:], af_f[:, b:b + 1])
                    nc.vector.tensor_mul(w[:], w[:], bf_rep[:, jl])
                    nc.sync.dma_start(out_d[:, sl], w[:])
    nc.compile()
    return nc


def kernel(AT, k, bt, _trace=False):
    global LAST_RESULT
    from concourse.bass_utils import run_bass_kernel_spmd

    assert AT.shape == (NA,) and k.shape == (NA, NB) and bt.shape == (NB,)
    K = (np.asarray(k, np.float32) * np.asarray(k, np.float32))
    AT = np.asarray(AT, np.float32)
    BT2 = np.asarray(bt, np.float32) * np.asarray(bt, np.float32)

    if "nc" not in _CACHE:
        _CACHE["nc"] = _build()
    nc = _CACHE["nc"]

    in_maps = []
    for m in range(N_CORES):
        rows = K[m * L:(m + 1) * L]                      # [512, NB]
        krows = np.ascontiguousarray(rows.reshape(128, 4 * NB))
        kT = np.ascontiguousarray(
            rows.reshape(L, 128, 32).transpose(1, 2, 0).reshape(128, 32 * L))
        in_maps.append({
            "krows": krows,
            "kcolsT": kT,
            "kf32": krows,
            "at_sb": np.ascontiguousarray(AT[m * L:(m + 1) * L].reshape(128, 4)),
            "bt2_sb": np.ascontiguousarray(BT2.reshape(128, 32)),
        })

    res = run_bass_kernel_spmd(nc, in_maps, core_ids=list(range(N_CORES)),
                               trace=_trace)
    LAST_RESULT = res

    C = np.empty((NA, NB), np.float32)
    for m in range(N_CORES):
        C[m * L:(m + 1) * L] = res.results[m]["cout"].reshape(L, NB)
    return C
